# revision 1
# baseline (speedup 1.0000x reference)
"""Trainium2 Bass kernel for a 16-layer fully-connected chain (matvec per layer).

Computation (reference):
    v = x                       # [2048]
    for i in 0..13:  v = silu(W[i] @ v + b[i])
    out = W[14] @ v + b[14]

Strategy (8 NeuronCores):
  - Row-shard every layer: core c computes output neurons [c*256, (c+1)*256).
    Weights are streamed from HBM (252 MB total / 31.5 MB per core) — this is
    the memory-bound resource, split 8 ways.
  - The per-core matvec runs on the TensorEngine with the weight slice as the
    MOVING operand (rhs, [k=128, m=256] float32r tiles) and the activation vector
    as the stationary operand (lhsT, [k=128, 1]), accumulating over 16 k-tiles
    into PSUM [1, 256]. The bias is folded in as an extra rank-1 matmul
    (ones[1,1] x bias[1,256]).
  - silu on the ScalarEngine, then an AllGather over all 8 cores rebuilds the
    full activation vector for the next layer. The last layer needs no
    gather: each core returns its 256-slice and the host concatenates.
  - Weight layout is prepared on the host so each layer's per-core slice is
    one fully contiguous 2 MB HBM->SBUF DMA, and the gathered activation
    reloads as [128, 16] with 64 contiguous bytes per partition
    (k-index = p*16 + t, baked into the host-side weight permutation).
"""

import numpy as np

_L = 15        # number of weight matrices
_N = 2048      # neurons per layer
_M = 8         # cores
_SH = _N // _M  # 256 output slice per core
_KT = _N // 128  # 16 k-tiles

_CACHE = {}


def _build(mm_dtype="float32r", act="Silu"):
    import concourse.bacc as bacc
    import concourse.mybir as mybir
    import concourse.tile as tile

    f32 = mybir.dt.float32
    mdt = getattr(mybir.dt, mm_dtype)

    nc = bacc.Bacc("TRN2", target_bir_lowering=False, debug=False,
                   num_devices=_M)

    wt = nc.dram_tensor("wt", [_L, 128, _KT * _SH], mdt, kind="ExternalInput")
    # bias slices for all layers + a trailing constant 1.0 (used as the
    # rank-1 stationary operand that folds the bias add into the PSUM group)
    bias = nc.dram_tensor("bias", [1, _L * _SH + 1], mdt, kind="ExternalInput")
    x0 = nc.dram_tensor("x0", [128, _KT], mdt, kind="ExternalInput")
    out = nc.dram_tensor("out", [1, _SH], f32, kind="ExternalOutput")

    with tile.TileContext(nc) as tc:
        with (
            tc.tile_pool(name="w", bufs=3) as wpool,
            tc.tile_pool(name="v", bufs=2) as vpool,
            tc.tile_pool(name="s", bufs=2) as spool,
            tc.tile_pool(name="consts", bufs=1) as cpool,
            tc.tile_pool(name="ps", bufs=2, space="PSUM") as pspool,
            tc.tile_pool(name="dram", bufs=3, space="DRAM") as dpool,
        ):
            bias_t = cpool.tile([1, _L * _SH + 1], mdt)
            nc.sync.dma_start(bias_t[:], bias.ap())
            ones_t = bias_t[:, _L * _SH:_L * _SH + 1]

            v = vpool.tile([128, _KT], mdt, tag="v")
            nc.sync.dma_start(v[:], x0.ap())

            for i in range(_L):
                w = wpool.tile([128, _KT * _SH], mdt, tag="w")
                nc.sync.dma_start(w[:], wt.ap()[i])

                ps = pspool.tile([1, _SH], f32, tag="ps")
                for t in range(_KT):
                    nc.tensor.matmul(
                        ps[:],
                        lhsT=v[:, t:t + 1],
                        rhs=w[:, t * _SH:(t + 1) * _SH],
                        start=(t == 0),
                        stop=False,
                    )
                nc.tensor.matmul(
                    ps[:],
                    lhsT=ones_t,
                    rhs=bias_t[:, i * _SH:(i + 1) * _SH],
                    start=False,
                    stop=True,
                )

                if i < _L - 1:
                    s = spool.tile([1, _SH], mdt, tag="s")
                    nc.scalar.activation(
                        s[:], ps[:],
                        getattr(mybir.ActivationFunctionType, act))
                    cc_in = dpool.tile([1, _SH], mdt, tag="ccin")
                    nc.sync.dma_start(cc_in[:], s[:])
                    cc_out = dpool.tile([1, _N], mdt, tag="ccout")
                    nc.gpsimd.collective_compute(
                        "AllGather",
                        mybir.AluOpType.bypass,
                        replica_groups=[list(range(_M))],
                        ins=[cc_in.opt()],
                        outs=[cc_out.opt()],
                    )
                    v = vpool.tile([128, _KT], mdt, tag="v")
                    nc.sync.dma_start(
                        v[:], cc_out[0, :].rearrange("(p t) -> p t", p=128))
                else:
                    s = spool.tile([1, _SH], f32, tag="sout")
                    nc.vector.tensor_copy(s[:], ps[:])
                    nc.sync.dma_start(out.ap(), s[:])

    nc.compile()
    return nc


def _prep_inputs(x, W, b):
    """Host-side sharding/layout prep. k-index (p, t): k = p*16 + t."""
    W = np.ascontiguousarray(W, dtype=np.float32)
    # W[i, m, k] with m = (c, j), k = (p, t)
    Wv = W.reshape(_L, _M, _SH, 128, _KT)
    # -> [c, i, p, t, j]
    Wc = Wv.transpose(1, 0, 3, 4, 2).reshape(_M, _L, 128, _KT * _SH)
    x0 = np.ascontiguousarray(
        np.asarray(x, dtype=np.float32).reshape(128, _KT))
    in_maps = []
    for c in range(_M):
        in_maps.append({
            "wt": np.ascontiguousarray(Wc[c]),
            "bias": np.ascontiguousarray(np.concatenate([
                np.asarray(b[:, c * _SH:(c + 1) * _SH],
                           dtype=np.float32).reshape(-1),
                np.ones(1, dtype=np.float32),
            ]).reshape(1, _L * _SH + 1)),
            "x0": x0,
        })
    return in_maps


def kernel(x, W, b, _trace=False):
    from concourse.bass_utils import run_bass_kernel_spmd

    key = "nc"
    if key not in _CACHE:
        _CACHE[key] = _build()
    nc = _CACHE[key]

    in_maps = _prep_inputs(x, W, b)
    res = run_bass_kernel_spmd(
        nc, in_maps, core_ids=list(range(_M)), trace=_trace)
    _CACHE["last_results"] = res
    return np.concatenate([res.results[c]["out"][0] for c in range(_M)])



# revision 2
# speedup vs baseline: 1.2442x; 1.2442x over previous
"""Trainium2 Bass kernel for a 16-layer fully-connected chain (matvec per layer).

Computation (reference):
    v = x                       # [2048]
    for i in 0..13:  v = silu(W[i] @ v + b[i])
    out = W[14] @ v + b[14]

Strategy (8 NeuronCores):
  - Row-shard every layer: core c computes output neurons [c*256, (c+1)*256).
  - fp16 weights/activations with a per-layer power-of-4 rescaling so the
    growing activations (up to ~1e8) stay in fp16 range:
        vt_i  = v_i / 4^i        (the tensors that move through PE/collective)
        Wt_i  = W_i / 4 (i<14),  W_14 unscaled
        bt_i  = b_i / 4^(i+1)    (b_14 / 4^14)
    PSUM then holds  p = (W_i @ v_i + b_i) / 4^(i+1)  in fp32; the scalar
    engine computes v_true = silu(p * 4^(i+1)) and vt_{i+1} = v_true / 4^(i+1)
    (two activation ops).  Verified vs fp32 reference: rel err ~2e-3.
  - All 15 per-core weight slices (15 x 1 MB fp16) are resident in SBUF;
    they stream in up-front on the Sync-engine HWDGE queue with no
    dependencies, fully overlapped with the compute/collective chain.
  - The latency-critical small DMAs (activation bounce to DRAM for the
    AllGather, and the gathered-vector reload) go on the Scalar-engine
    HWDGE queue.  HWDGE completion counting-semaphores (DMAHW0..7) are
    assigned round-robin over ALL HWDGE dma_starts in issue order, so the
    python-level issue order below strictly alternates weight-class DMAs
    (even lanes) with small-class DMAs (odd lanes).  Without this, the
    collective trigger waits on a lane whose count includes a 1 MB weight
    prefetch, adding ~8 us of dead time per layer.
  - Per layer: bias matmul first (start=True, independent of v), then 16
    k-tile matmuls (lhsT = vt column, moving = 128x256 fp16 weight tile),
    silu+rescale on scalar, bounce DMA, AllGather (8 ranks), reload.
"""

import numpy as np

_L = 15        # number of weight matrices
_N = 2048      # neurons per layer
_M = 8         # cores
_SH = _N // _M  # 256 output slice per core
_KT = _N // 128  # 16 k-tiles

_CACHE = {}


def _build():
    import concourse.bacc as bacc
    import concourse.mybir as mybir
    import concourse.tile as tile

    f32 = mybir.dt.float32
    f16 = mybir.dt.float16
    AF = mybir.ActivationFunctionType

    nc = bacc.Bacc("TRN2", target_bir_lowering=False, debug=False,
                   num_devices=_M)

    wt = nc.dram_tensor("wt", [_L, 128, _KT * _SH], f16, kind="ExternalInput")
    # bias rows for all layers + a trailing constant 1.0 (rank-1 stationary
    # operand folding the bias add into the PSUM group)
    bias = nc.dram_tensor("bias", [1, _L * _SH + 1], f16, kind="ExternalInput")
    x0 = nc.dram_tensor("x0", [128, _KT], f16, kind="ExternalInput")
    out = nc.dram_tensor("out", [1, _SH], f32, kind="ExternalOutput")

    HK = _KT * _SH // 2  # half a layer's free size (2048 cols)

    with tile.TileContext(nc) as tc:
        with (
            tc.tile_pool(name="w", bufs=1) as wpool,
            tc.tile_pool(name="v", bufs=2) as vpool,
            tc.tile_pool(name="s", bufs=2) as spool,
            tc.tile_pool(name="consts", bufs=1) as cpool,
            tc.tile_pool(name="ps", bufs=2, space="PSUM") as pspool,
            tc.tile_pool(name="dram", bufs=2, space="DRAM") as dpool,
        ):
            # HWDGE lane parity: idx0 bias (even), idx1 x0 (odd), then the
            # loop contributes exactly 4 per iteration: w_a (even),
            # small (odd), w_b (even), small (odd).
            bias_t = cpool.tile([1, _L * _SH + 1], f16)
            nc.sync.dma_start(bias_t[:], bias.ap())
            ones_t = bias_t[:, _L * _SH:_L * _SH + 1]

            v = vpool.tile([128, _KT], f16, tag="v")
            nc.sync.dma_start(v[:], x0.ap())

            scratch = cpool.tile([1, _KT], f16)

            w_tiles = []
            s16_prev = None  # previous layer's fp16 activation slice
            for i in range(_L):
                w = wpool.tile([128, _KT * _SH], f16, tag=f"w{i}")
                w_tiles.append(w)
                # weight half A (even lane)
                nc.sync.dma_start(w[:, :HK], wt.ap()[i, :, :HK])
                # small DMA (odd lane): bounce of previous layer's act
                if i == 0:
                    nc.scalar.dma_start(scratch[:], x0.ap()[0:1, :])
                else:
                    cc_in = dpool.tile([1, _SH], f16, tag="ccin")
                    nc.scalar.dma_start(cc_in[:], s16_prev[:])
                # weight half B (even lane)
                nc.sync.dma_start(w[:, HK:], wt.ap()[i, :, HK:])
                # small DMA (odd lane): reload of gathered activations
                if i == 0:
                    nc.scalar.dma_start(scratch[:], x0.ap()[0:1, :])
                else:
                    cc_out = dpool.tile([1, _N], f16, tag="ccout")
                    nc.gpsimd.collective_compute(
                        "AllGather",
                        mybir.AluOpType.bypass,
                        replica_groups=[list(range(_M))],
                        ins=[cc_in.opt()],
                        outs=[cc_out.opt()],
                    )
                    v = vpool.tile([128, _KT], f16, tag="v")
                    nc.scalar.dma_start(
                        v[:], cc_out[0, :].rearrange("(p t) -> p t", p=128))

                ps = pspool.tile([1, _SH], f32, tag="ps")
                # bias first: independent of v, runs during the AllGather
                nc.tensor.matmul(
                    ps[:],
                    lhsT=ones_t,
                    rhs=bias_t[:, i * _SH:(i + 1) * _SH],
                    start=True,
                    stop=False,
                )
                for t in range(_KT):
                    nc.tensor.matmul(
                        ps[:],
                        lhsT=v[:, t:t + 1],
                        rhs=w[:, t * _SH:(t + 1) * _SH],
                        start=False,
                        stop=(t == _KT - 1),
                    )

                if i < _L - 1:
                    # v_true = silu(p * 4^(i+1));  vt = v_true / 4^(i+1)
                    sc = float(4.0 ** (i + 1))
                    s32 = spool.tile([1, _SH], f32, tag="s32")
                    nc.scalar.activation(s32[:], ps[:], AF.Silu, scale=sc)
                    s16 = spool.tile([1, _SH], f16, tag="s16")
                    nc.scalar.activation(s16[:], s32[:], AF.Copy,
                                         scale=1.0 / sc)
                    s16_prev = s16
                else:
                    sout = spool.tile([1, _SH], f32, tag="sout")
                    nc.scalar.activation(sout[:], ps[:], AF.Copy,
                                         scale=float(4.0 ** 14))
                    nc.scalar.dma_start(out.ap(), sout[:])

    nc.compile()
    return nc


def _prep_inputs(x, W, b):
    """Host-side sharding/layout/scaling prep. k-index (p, t): k = p*16 + t."""
    W = np.asarray(W, dtype=np.float32)
    b = np.asarray(b, dtype=np.float32)
    x = np.asarray(x, dtype=np.float32)

    Ws = W.copy()
    Ws[:_L - 1] *= 0.25
    W16 = Ws.astype(np.float16)
    # W[i, m, k] with m = (c, j), k = (p, t)
    Wv = W16.reshape(_L, _M, _SH, 128, _KT)
    # -> [c, i, p, t, j]
    Wc = np.ascontiguousarray(Wv.transpose(1, 0, 3, 4, 2)).reshape(
        _M, _L, 128, _KT * _SH)

    scales = np.array([4.0 ** (i + 1) for i in range(_L - 1)] + [4.0 ** 14],
                      dtype=np.float32)
    bs = (b / scales[:, None]).astype(np.float16)

    x16 = np.ascontiguousarray(x.astype(np.float16).reshape(128, _KT))
    in_maps = []
    for c in range(_M):
        brow = np.concatenate([
            bs[:, c * _SH:(c + 1) * _SH].reshape(-1),
            np.ones(1, dtype=np.float16),
        ]).reshape(1, _L * _SH + 1)
        in_maps.append({
            "wt": np.ascontiguousarray(Wc[c]),
            "bias": np.ascontiguousarray(brow),
            "x0": x16,
        })
    return in_maps


def kernel(x, W, b, _trace=False):
    from concourse.bass_utils import run_bass_kernel_spmd

    key = "nc"
    if key not in _CACHE:
        _CACHE[key] = _build()
    nc = _CACHE[key]

    in_maps = _prep_inputs(x, W, b)
    res = run_bass_kernel_spmd(
        nc, in_maps, core_ids=list(range(_M)), trace=_trace)
    _CACHE["last_results"] = res
    return np.concatenate([res.results[c]["out"][0] for c in range(_M)])


# revision 9
# speedup vs baseline: 1.3022x; 1.0466x over previous
"""Trainium2 Bass kernel for a 16-layer fully-connected chain (matvec per layer).

Computation (reference):
    v = x                       # [2048]
    for i in 0..13:  v = silu(W[i] @ v + b[i])
    out = W[14] @ v + b[14]

Strategy (8 NeuronCores):
  - Row-shard every layer: core c computes output neurons [c*256, (c+1)*256).
  - fp16 weights/activations with a per-layer power-of-4 rescaling so the
    growing activations (up to ~1e8) stay in fp16 range:
        vt_i  = v_i / 4^i        (the tensors that move through PE/collective)
        Wt_i  = W_i / 4 (i<14),  W_14 unscaled
        bt_i  = b_i / 4^(i+1)    (b_14 / 4^14)
    PSUM then holds  p = (W_i @ v_i + b_i) / 4^(i+1)  in fp32; the scalar
    engine computes v_true = silu(p * 4^(i+1)) and vt_{i+1} = v_true / 4^(i+1)
    (two activation ops).  Verified vs fp32 reference: rel err ~2e-3.
  - All 15 per-core weight slices (15 x 1 MB fp16) are resident in SBUF;
    they stream in up-front on the Sync-engine HWDGE queue with no
    dependencies, fully overlapped with the compute/collective chain.
  - The latency-critical small DMAs (activation bounce to DRAM for the
    AllGather, and the gathered-vector reload) go on the Scalar-engine
    HWDGE queue.  HWDGE completion counting-semaphores (DMAHW0..7) are
    assigned round-robin over ALL HWDGE dma_starts in issue order, so the
    python-level issue order below strictly alternates weight-class DMAs
    (even lanes) with small-class DMAs (odd lanes).  Without this, the
    collective trigger waits on a lane whose count includes a 1 MB weight
    prefetch, adding ~8 us of dead time per layer.
  - Per layer: bias matmul first (start=True, independent of v), then 16
    k-tile matmuls (lhsT = vt column, moving = 128x256 fp16 weight tile),
    silu+rescale on scalar, bounce DMA, AllGather (8 ranks), reload.
"""

import numpy as np

_L = 15        # number of weight matrices
_N = 2048      # neurons per layer
_M = 8         # cores
_SH = _N // _M  # 256 output slice per core
_KT = _N // 128  # 16 k-tiles

_CACHE = {}


def _build():
    import concourse.bacc as bacc
    import concourse.mybir as mybir
    import concourse.tile as tile

    f32 = mybir.dt.float32
    f16 = mybir.dt.float16
    AF = mybir.ActivationFunctionType

    nc = bacc.Bacc("TRN2", target_bir_lowering=False, debug=False,
                   num_devices=_M)

    wt = nc.dram_tensor("wt", [_L, 128, _KT * _SH], f16, kind="ExternalInput")
    # bias rows for all layers + a trailing constant 1.0 (rank-1 stationary
    # operand folding the bias add into the PSUM group)
    bias = nc.dram_tensor("bias", [1, _L * _SH + 1], f16, kind="ExternalInput")
    x0 = nc.dram_tensor("x0", [128, _KT], f16, kind="ExternalInput")
    out = nc.dram_tensor("out", [1, _SH], f32, kind="ExternalOutput")

    HK = _KT * _SH // 2  # half a layer's free size (2048 cols)

    with tile.TileContext(nc) as tc:
        with (
            tc.tile_pool(name="w", bufs=1) as wpool,
            tc.tile_pool(name="v", bufs=2) as vpool,
            tc.tile_pool(name="s", bufs=2) as spool,
            tc.tile_pool(name="consts", bufs=1) as cpool,
            tc.tile_pool(name="ps", bufs=4, space="PSUM") as pspool,
            tc.tile_pool(name="dram", bufs=2, space="DRAM") as dpool,
        ):
            # HWDGE lane parity: idx0 bias (even), idx1 x0 (odd), then the
            # loop contributes exactly 4 per iteration: w_a (even),
            # small (odd), w_b (even), small (odd).
            bias_t = cpool.tile([1, _L * _SH + 1], f16)
            nc.sync.dma_start(bias_t[:], bias.ap())
            ones_t = bias_t[:, _L * _SH:_L * _SH + 1]

            v = vpool.tile([128, _KT], f16, tag="v")
            nc.sync.dma_start(v[:], x0.ap())

            scratch = cpool.tile([1, _KT], f16)
            scratch2 = cpool.tile([1, _KT], f16)

            w_tiles = []
            s16_prev = None  # previous layer's fp16 activation slice
            for i in range(_L):
                w = wpool.tile([128, _KT * _SH], f16, tag=f"w{i}")
                w_tiles.append(w)
                # weight half A (even lane)
                nc.sync.dma_start(w[:, :HK], wt.ap()[i, :, :HK])
                # small DMA (odd lane): bounce of previous layer's act
                if i == 0:
                    nc.scalar.dma_start(scratch[:], x0.ap()[0:1, :])
                else:
                    cc_in = dpool.tile([1, _SH], f16, tag="ccin")
                    nc.scalar.dma_start(cc_in[:], s16_prev[:],
                                        single_packet=True)
                # weight half B (even lane)
                nc.sync.dma_start(w[:, HK:], wt.ap()[i, :, HK:])
                # small DMAs (odd lanes), with an instant sync-queue dummy
                # in between to keep the even/odd lane parity: reload of
                # the gathered activations, split so the hi half's DMA +
                # semaphore latency hides under the first 8 k-tile matmuls
                if i == 0:
                    nc.scalar.dma_start(scratch[:], x0.ap()[0:1, :])
                    nc.sync.dma_start(scratch2[:], x0.ap()[0:1, :])
                    nc.scalar.dma_start(scratch[:], x0.ap()[0:1, :])
                else:
                    cc_out = dpool.tile([1, _N], f16, tag="ccout")
                    nc.gpsimd.collective_compute(
                        "AllGather",
                        mybir.AluOpType.bypass,
                        replica_groups=[list(range(_M))],
                        ins=[cc_in.opt()],
                        outs=[cc_out.opt()],
                    )
                    v = vpool.tile([128, _KT], f16, tag="v")
                    cc_v = cc_out[0, :].rearrange("(p t) -> p t", p=128)
                    nc.scalar.dma_start(v[:, 0:_KT // 2],
                                        cc_v[:, 0:_KT // 2],
                                        single_packet=True)
                    nc.sync.dma_start(scratch2[:], x0.ap()[0:1, :])
                    nc.scalar.dma_start(v[:, _KT // 2:],
                                        cc_v[:, _KT // 2:],
                                        single_packet=True)

                ps = pspool.tile([1, _SH], f32, tag="ps")
                # bias first: independent of v, runs during the AllGather
                nc.tensor.matmul(
                    ps[:],
                    lhsT=ones_t,
                    rhs=bias_t[:, i * _SH:(i + 1) * _SH],
                    start=True,
                    stop=False,
                )
                for t in range(_KT):
                    nc.tensor.matmul(
                        ps[:],
                        lhsT=v[:, t:t + 1],
                        rhs=w[:, t * _SH:(t + 1) * _SH],
                        start=False,
                        stop=(t == _KT - 1),
                    )

                if i < _L - 1:
                    # v_true = silu(p * 4^(i+1));  vt = v_true / 4^(i+1)
                    sc = float(4.0 ** (i + 1))
                    s32 = spool.tile([1, _SH], f32, tag="s32")
                    nc.scalar.activation(s32[:], ps[:], AF.Silu, scale=sc)
                    s16 = spool.tile([1, _SH], f16, tag="s16")
                    nc.scalar.activation(s16[:], s32[:], AF.Copy,
                                         scale=1.0 / sc)
                    s16_prev = s16
                else:
                    sout = spool.tile([1, _SH], f32, tag="sout")
                    nc.scalar.activation(sout[:], ps[:], AF.Copy,
                                         scale=float(4.0 ** 14))
                    nc.scalar.dma_start(out.ap(), sout[:],
                                        single_packet=True)

    nc.compile()
    return nc


def _prep_inputs(x, W, b):
    """Host-side sharding/layout/scaling prep. k-index (p, t): k = p*16 + t."""
    W = np.asarray(W, dtype=np.float32)
    b = np.asarray(b, dtype=np.float32)
    x = np.asarray(x, dtype=np.float32)

    Ws = W.copy()
    Ws[:_L - 1] *= 0.25
    W16 = Ws.astype(np.float16)
    # W[i, m, k] with m = (c, j), k = (p, t)
    Wv = W16.reshape(_L, _M, _SH, 128, _KT)
    # -> [c, i, p, t, j]
    Wc = np.ascontiguousarray(Wv.transpose(1, 0, 3, 4, 2)).reshape(
        _M, _L, 128, _KT * _SH)

    scales = np.array([4.0 ** (i + 1) for i in range(_L - 1)] + [4.0 ** 14],
                      dtype=np.float32)
    bs = (b / scales[:, None]).astype(np.float16)

    x16 = np.ascontiguousarray(x.astype(np.float16).reshape(128, _KT))
    in_maps = []
    for c in range(_M):
        brow = np.concatenate([
            bs[:, c * _SH:(c + 1) * _SH].reshape(-1),
            np.ones(1, dtype=np.float16),
        ]).reshape(1, _L * _SH + 1)
        in_maps.append({
            "wt": np.ascontiguousarray(Wc[c]),
            "bias": np.ascontiguousarray(brow),
            "x0": x16,
        })
    return in_maps


def kernel(x, W, b, _trace=False):
    from concourse.bass_utils import run_bass_kernel_spmd

    key = "nc"
    if key not in _CACHE:
        _CACHE[key] = _build()
    nc = _CACHE[key]

    in_maps = _prep_inputs(x, W, b)
    res = run_bass_kernel_spmd(
        nc, in_maps, core_ids=list(range(_M)), trace=_trace)
    _CACHE["last_results"] = res
    return np.concatenate([res.results[c]["out"][0] for c in range(_M)])
